# revision 34
# baseline (speedup 1.0000x reference)
"""Trainium2 Bass kernel for CombinedBandPassFilterSequential.

Zero-phase (filtfilt-style) FIR filter bank: 10 phase bands (K=769) +
10 amplitude bands (K=129) over a single (1,1,2097152) fp32 signal;
output is the 20 band signals concatenated on the last axis.

Strategy
--------
Time-sharded SPMD over 8 NeuronCores: each core processes a contiguous
T/8 slice of the signal for ALL 20 bands (perfect load balance).

The two-pass zero-phase filter equals a single cross-correlation with
g = autocorr(h) (2K-1 taps) everywhere except the first/last (K-1)/2
samples of the GLOBAL sequence; those few samples are computed exactly
on the host (numpy, float64) and spliced into the result. This fuses
the reference's two conv passes into one dense pass with no
intermediate staging and no edge masks on the device.

Each 1-D correlation is cast as a sequence of 128x128 @ 128x512
tensor-engine matmuls using banded-Toeplitz weight chunks
(PSUM accumulates fp32):

  out[128*i + r] = sum_q  W_q[:, r] . x_cols[:, i + q - Q0]

where x_cols[p, m] = x[128*m + p] is the signal in "transposed" column
layout (prepared on host) and W_q[p, r] = g[128*(q-Q0) + p - r + c].

Mixed precision, tuned so worst-case rel err stays under the 2e-2
gate (measured 1.655e-2, deterministic; validated bit-exactly by a
host-side quantization simulator): inner taps run in fp16 (1 col/cycle
on the PE, 4x less quantization noise than bf16 for free); outer taps
run as fp8e4 DoubleRow pairs (256-wide contraction per instruction =
2 chunks per ~1.1*512 cycles, ~1.9x fp16 throughput). pha: 5 fp16
chunks (q' in [-2,2]) + 4 fp8 pairs ((-6,-5),(-4,-3),(+3,+4),(+5,+6));
amp: 1 fp16 chunk (q'=+1) + 1 fp8 pair ((-1,0)). Every DR pair is
served by ONE shift-1 stacked fp8 signal copy (xd8), so the only
device inputs are xt (fp16 signal), xd8, and the weight chunks.
Output staged/stored fp16.

Schedule (all measured on HW traces): ~48 zero-weight N=128 warmup
matmuls (no DMA deps) keep the PE busy through the HAM activity window
during the initial DMA wait so real matmuls run at 2.4 GHz from the
start; the three DMA rings (sync=xt, scalar=weights, gpsimd=xd8) each
lead with exactly what the first matmuls need (the rings share ~350
GB/s, and per-DMA costs ~0.65us sequencer issue + ~2us completion
latency, so band weights go as per-band slices in consumption order);
PSUM drains alternate vector/scalar into a per-band fp16 stage stored
as one 512KB DMA. The final band tapers its last groups (512,512,512,
256,128,128 cols) with parallel half-copies and sync/scalar-alternating
stores so the last store's copy+issue+data+sem chain is minimal; the
gpsimd ring gets no late DMAs (its end-of-program drain takes ~3.5us).
"""
import numpy as np
import ml_dtypes

import concourse.bass as bass
import concourse.tile as tile
from concourse import bacc, mybir
from concourse import bass_utils

# ---- problem geometry (hardcoded per contest rules) ----
T = 2097152
NCORES = 8
L = T // NCORES          # 262144 samples per core
LC = L // 128            # 2048 output columns per core
XH = 6                   # x halo columns each side (= pha (K-1)/128)
XC = LC + 2 * XH         # 2060 x columns
NB = 10                  # bands per filter group
QP, Q0P = 13, 6          # pha fused autocorr (1537 taps): chunk count, offset
QA, Q0A = 3, 1           # amp fused autocorr (257 taps): chunk count, offset
PB = 5                   # pha fp16 chunks per band (q' in [-2, 2])
NP8 = 8                  # pha fp8 chunks per band (q' = -6..-3, +3..+6)
P8OFF = (0, 2, 9, 11)    # xd8 base offsets of the 4 fp8 pairs
CP = 384                 # pha edge-splice width ((K-1)/2)
CA = 64                  # amp edge-splice width
N = 512                  # matmul moving width (1 PSUM bank)
NG = LC // N             # 4 groups per band
NDUMMY = 48              # PE warmup matmuls (N=128)

F32 = mybir.dt.float32
FP16 = mybir.dt.float16
FP8 = mybir.dt.float8e4
FP16_NP = np.float16
FP8_NP = ml_dtypes.float8_e4m3
DR = mybir.MatmulPerfMode.DoubleRow


def _toeplitz_chunks(g, Q0, NQ):
    """W[q][p, r] = g[128*(q - Q0) + p - r + c], zero outside [0, len(g))."""
    g = np.asarray(g, np.float64)
    K = len(g)
    c = (K - 1) // 2
    W = np.zeros((NQ, 128, 128), np.float64)
    p = np.arange(128)[:, None]
    r = np.arange(128)[None, :]
    for q in range(NQ):
        k = 128 * (q - Q0) + p - r + c
        valid = (k >= 0) & (k < K)
        W[q][valid] = g[np.clip(k, 0, K - 1)][valid]
    return W


def _build_program():
    nc = bacc.Bacc("TRN2", target_bir_lowering=False, debug=False,
                   enable_asserts=False, num_devices=NCORES)

    x_ap = nc.dram_tensor("xT", [128, XC], FP16, kind="ExternalInput").ap()
    xd_ap = nc.dram_tensor("xd8", [128, 2, XC], FP8, kind="ExternalInput").ap()
    wp_ap = nc.dram_tensor("wp", [128, NB * PB * 128], FP16,
                           kind="ExternalInput").ap()
    wp8_ap = nc.dram_tensor("wp8", [128, NB * NP8, 128], FP8,
                            kind="ExternalInput").ap()
    wa_ap = nc.dram_tensor("wa", [128, NB * 128], FP16,
                           kind="ExternalInput").ap()
    wa8_ap = nc.dram_tensor("wa8", [128, NB * 2, 128], FP8,
                            kind="ExternalInput").ap()
    out_ap = nc.dram_tensor("out", [2 * NB, 128, LC], FP16,
                            kind="ExternalOutput").ap()

    with tile.TileContext(nc) as tc:
        with tc.tile_pool(name="const", bufs=1) as cpool, \
             tc.tile_pool(name="psum", bufs=8, space="PSUM") as psum_pool, \
             tc.tile_pool(name="stage", bufs=8) as stage_pool:

            xt = cpool.tile([128, XC], FP16, name="xt", tag="xT")
            xd8 = cpool.tile([128, 2, XC], FP8, name="xdt", tag="xd8")
            wp = cpool.tile([128, NB * PB * 128], FP16, name="wpt", tag="wp")
            wp8 = cpool.tile([128, NB * NP8, 128], FP8, name="wp8t", tag="wp8")
            wa = cpool.tile([128, NB * 128], FP16, name="wat", tag="wa")
            wa8 = cpool.tile([128, NB * 2, 128], FP8, name="wa8t", tag="wa8")
            zt = cpool.tile([128, 128], FP16, name="zt", tag=None)

            # PE warmup: zero-weight matmuls with no DMA dependencies keep
            # the PE busy through the HAM activity window during the
            # startup DMA wait, so real matmuls run at 2.4 GHz from the
            # first instruction.
            nc.vector.memset(zt[:], 0.0)
            dps = psum_pool.tile([128, 512], F32, tag="ps")
            for _ in range(NDUMMY):
                nc.tensor.matmul(dps[:, 0:128], zt[:], zt[:],
                                 start=True, stop=True)

            # Startup-critical DMAs, one ring each so they land together.
            # The first real matmul (band-0 g0, fp16 chunk j=0) gates on
            # 33 KB of weights (scalar) + 133 KB of xt (sync); each ring
            # leads with exactly what the earliest matmuls need.
            nc.sync.dma_start(xt[:, 0:520], x_ap[:, 0:520])
            nc.scalar.dma_start(wp[:, 0:128], wp_ap[:, 0:128])
            nc.gpsimd.dma_start(xd8[:, :, 0:520], xd_ap[:, :, 0:520])
            nc.sync.dma_start(xt[:, 520:1040], x_ap[:, 520:1040])
            nc.scalar.dma_start(wp[:, 128:PB * 128], wp_ap[:, 128:PB * 128])
            nc.gpsimd.dma_start(xd8[:, :, 520:1040], xd_ap[:, :, 520:1040])
            nc.sync.dma_start(xt[:, 1040:1560], x_ap[:, 1040:1560])
            nc.scalar.dma_start(wp8[:, 0:NP8, :], wp8_ap[:, 0:NP8, :])
            nc.gpsimd.dma_start(xd8[:, :, 1040:1560], xd_ap[:, :, 1040:1560])
            nc.sync.dma_start(xt[:, 1560:XC], x_ap[:, 1560:XC])
            nc.scalar.dma_start(wa[:, 0:128], wa_ap[:, 0:128])
            nc.gpsimd.dma_start(xd8[:, :, 1560:XC], xd_ap[:, :, 1560:XC])
            nc.scalar.dma_start(wa8[:, 0:2, :], wa8_ap[:, 0:2, :])

            # bands 1-9 weights: per-band slices in consumption order so
            # each band's wait clears as soon as its own bytes land (the
            # input stream saturates DMA bandwidth for the first ~15us;
            # one big DMA would gate band 1 on band 9's bytes). Odd bands
            # ride the scalar ring, even bands gpsimd; fp8 before fp16
            # within a band pair matches nothing in-group (fp16 runs
            # first) but keeps the tighter-deadline DR weights earlier.
            def wband(eng, b):
                eng.dma_start(wp8[:, b * NP8:(b + 1) * NP8, :],
                              wp8_ap[:, b * NP8:(b + 1) * NP8, :])
                eng.dma_start(wp[:, b * PB * 128:(b + 1) * PB * 128],
                              wp_ap[:, b * PB * 128:(b + 1) * PB * 128])

            wband(nc.scalar, 1)
            nc.gpsimd.dma_start(wa[:, 128:], wa_ap[:, 128:])
            nc.gpsimd.dma_start(wa8[:, 2:, :], wa8_ap[:, 2:, :])
            for b in range(2, NB):
                wband(nc.scalar if b % 2 else nc.gpsimd, b)

            def pha_group(ps, b, i0, c0, w, first, stop):
                # one accumulation window over psum cols [c0, c0+w)
                for j in range(PB):
                    m0 = i0 + c0 + 4 + j
                    nc.tensor.matmul(
                        ps[:, c0:c0 + w],
                        wp[:, (b * PB + j) * 128:(b * PB + j + 1) * 128],
                        xt[:, m0:m0 + w],
                        start=(first and j == 0), stop=False)
                # fp8 pairs: (-6,-5), (-4,-3), (+3,+4), (+5,+6)
                for k, off in enumerate(P8OFF):
                    nc.tensor.matmul(
                        ps[:, c0:c0 + w],
                        wp8[:, b * NP8 + 2 * k:b * NP8 + 2 * k + 2, :],
                        xd8[:, :, i0 + c0 + off:i0 + c0 + off + w],
                        start=False, stop=(stop and k == 3), perf_mode=DR)

            def pha_band(b, last=False):
                if last:
                    # final band: taper the last groups so the very last
                    # store's copy+issue+data+sem chain is minimal; parallel
                    # half-copies, stores alternate sync/scalar rings (never
                    # gpsimd: its end-of-program queue drain takes ~3.5us).
                    widths = (N, N, N, N // 2, N // 4, N // 4)
                    i0 = 0
                    for g, w in enumerate(widths):
                        ps = psum_pool.tile([128, N], F32, tag="ps")
                        pha_group(ps, b, i0, 0, w, True, True)
                        st = stage_pool.tile([128, N], FP16, tag="stl")
                        h = w // 2
                        nc.vector.tensor_copy(st[:, 0:h], ps[:, 0:h])
                        nc.scalar.copy(st[:, h:w], ps[:, h:w])
                        eng = (nc.sync, nc.scalar)[g % 2]
                        eng.dma_start(out_ap[b, :, i0:i0 + w], st[:, 0:w])
                        i0 += w
                    return
                st = None
                for g in range(NG):
                    ps = psum_pool.tile([128, N], F32, tag="ps")
                    pha_group(ps, b, g * N, 0, N, True, True)
                    st = _drain(st, ps, g, b, False)

            def amp_group(b, i0, w):
                ps = psum_pool.tile([128, N], F32, tag="ps")
                # fp16 chunk q' = +1
                nc.tensor.matmul(
                    ps[:, 0:w], wa[:, b * 128:(b + 1) * 128],
                    xt[:, i0 + 7:i0 + 7 + w],
                    start=True, stop=False)
                # fp8 pair: chunks q' = -1, 0 (rhs blocks i-1, i) —
                # served by the same shift-1 stacked fp8 copy as pha
                nc.tensor.matmul(
                    ps[:, 0:w], wa8[:, b * 2:b * 2 + 2, :],
                    xd8[:, :, i0 + 5:i0 + 5 + w],
                    start=False, stop=True, perf_mode=DR)
                return ps

            def amp_band(b):
                st = None
                for g in range(NG):
                    ps = amp_group(b, g * N, N)
                    st = _drain(st, ps, g, NB + b, False)

            def _drain(st, ps, g, out_b, last):
                # all 4 groups of a band share one 2048-col fp16 stage ->
                # one 512KB store per band with 4KB DMA lines (the final
                # band is handled separately in amp_band)
                i0 = g * N
                if g == 0:
                    st = stage_pool.tile([128, LC], FP16, tag="st")
                if g % 2 == 0:
                    nc.vector.tensor_copy(st[:, i0:i0 + N], ps[:])
                else:
                    nc.scalar.copy(st[:, i0:i0 + N], ps[:])
                if g == NG - 1:
                    nc.sync.dma_start(out_ap[out_b, :, :], st[:])
                return st

            # amp interleaved between pha bands: amp's 2-matmul groups
            # produce drains ~4x faster than pha's 9-matmul groups; the
            # mix keeps DVE/ACT drain demand under their throughput. pha
            # runs first (its slow groups ride out the staggered xt/xd8
            # segment completion-sems at startup); the last pair is
            # swapped so the TAPERED pha band ends the program — its
            # widely-spaced drains leave the store rings idle, keeping
            # the final store chain off any queue.
            for b in range(NB - 1):
                pha_band(b)
                amp_band(b)
            amp_band(NB - 1)
            pha_band(NB - 1, last=True)

    nc.compile()
    return nc


_CACHE = {}


def _get_program():
    if "nc" not in _CACHE:
        _CACHE["nc"] = _build_program()
    return _CACHE["nc"]


def _host_inputs(x, pha_filters, amp_filters):
    x = np.ascontiguousarray(np.asarray(x, np.float32).reshape(T))
    pha = np.asarray(pha_filters, np.float64)
    amp = np.asarray(amp_filters, np.float64)

    gp = [np.correlate(h, h, "full") for h in pha]   # 1537 taps
    ga = [np.correlate(h, h, "full") for h in amp]   # 257 taps
    Wp = np.stack([_toeplitz_chunks(g, Q0P, QP) for g in gp])  # (NB,13,128,128)
    Wa = np.stack([_toeplitz_chunks(g, Q0A, QA) for g in ga])  # (NB,3,128,128)

    def wlay(W, dt):  # (NB, NQ, 128p, 128r) -> (128p, NB*NQ*128r)
        return np.ascontiguousarray(
            W.transpose(2, 0, 1, 3).reshape(128, -1).astype(dt))

    wp = wlay(Wp[:, 4:9], FP16_NP)                       # q' in [-2, 2]
    wp8 = wlay(Wp[:, [0, 1, 2, 3, 9, 10, 11, 12]], FP8_NP)  # q' = +-3..+-6
    wa = wlay(Wa[:, 2:3], FP16_NP)                       # q' = +1
    wa8 = wlay(Wa[:, [0, 1]], FP8_NP)                    # q' = -1, 0

    xpad = np.zeros(T + (2 * XH + 2) * 128, np.float32)
    xpad[XH * 128: XH * 128 + T] = x

    in_maps = []
    for c in range(NCORES):
        n0 = c * L
        xcols = xpad[n0:n0 + (XC + 2) * 128].reshape(XC + 2, 128).T
        xt = np.ascontiguousarray(xcols[:, :XC].astype(FP16_NP))
        x8 = xcols.astype(FP8_NP)
        xd8 = np.ascontiguousarray(
            np.stack([x8[:, 0:XC], x8[:, 1:XC + 1]], axis=1))
        in_maps.append({"xT": xt, "xd8": xd8,
                        "wp": wp, "wp8": wp8, "wa": wa, "wa8": wa8})
    return in_maps


def _edge_exact(x, h, W, win=3072):
    """Exact two-pass values for out[:W] and out[T-W:] (float64 host)."""
    K = len(h)
    c = (K - 1) // 2
    xs = x[:win]
    f1 = np.convolve(xs, h[::-1])
    y1 = f1[K - 1 - c:K - 1 - c + win]
    y1v = win - c
    f2 = np.convolve(y1[:y1v], h)
    head = f2[c:c + y1v - c][:W]
    xs = x[::-1][:win]
    f1 = np.convolve(xs, h)
    y1r = f1[c:c + win]
    f2 = np.convolve(y1r[:y1v], h[::-1])
    tail = f2[K - 1 - c:K - 1 - c + y1v - c][:W][::-1]
    return head, tail


def _gather(results, x, pha_filters, amp_filters):
    out = np.empty((2 * NB, T), np.float32)
    for c in range(NCORES):
        oc = np.asarray(results[c]["out"]).astype(np.float32)
        out[:, c * L:(c + 1) * L] = oc.transpose(0, 2, 1).reshape(2 * NB, L)
    # splice exact global-edge samples (fused autocorr differs from the
    # reference's cropped two-pass only within (K-1)/2 of each end)
    x64 = np.asarray(x, np.float64).reshape(T)
    for b in range(NB):
        head, tail = _edge_exact(x64, np.asarray(pha_filters[b], np.float64), CP)
        out[b, :CP] = head
        out[b, T - CP:] = tail
        head, tail = _edge_exact(x64, np.asarray(amp_filters[b], np.float64), CA)
        out[NB + b, :CA] = head
        out[NB + b, T - CA:] = tail
    return out.reshape(1, 1, 2 * NB * T)


def run(x, pha_filters, amp_filters, trace=False):
    nc = _get_program()
    in_maps = _host_inputs(x, pha_filters, amp_filters)
    res = bass_utils.run_bass_kernel_spmd(
        nc, in_maps, core_ids=list(range(NCORES)), trace=trace)
    return _gather(res.results, x, pha_filters, amp_filters), res


def kernel(x, pha_filters, amp_filters):
    out, _ = run(x, pha_filters, amp_filters)
    return out


# revision 35
# speedup vs baseline: 1.1940x; 1.1940x over previous
"""Trainium2 Bass kernel for CombinedBandPassFilterSequential.

Zero-phase (filtfilt-style) FIR filter bank: 10 phase bands (K=769) +
10 amplitude bands (K=129) over a single (1,1,2097152) fp32 signal;
output is the 20 band signals concatenated on the last axis.

Strategy
--------
Time-sharded SPMD over 8 NeuronCores: each core processes a contiguous
T/8 slice of the signal for ALL 20 bands (perfect load balance).

The two-pass zero-phase filter equals a single cross-correlation with
g = autocorr(h) (2K-1 taps) everywhere except the first/last (K-1)/2
samples of the GLOBAL sequence; those few samples are computed exactly
on the host (numpy, float64) and spliced into the result. This fuses
the reference's two conv passes into one dense pass with no
intermediate staging and no edge masks on the device.

Each 1-D correlation is cast as a sequence of 128x128 @ 128x512
tensor-engine matmuls using banded-Toeplitz weight chunks
(PSUM accumulates fp32):

  out[128*i + r] = sum_q  W_q[:, r] . x_cols[:, i + q - Q0]

where x_cols[p, m] = x[128*m + p] is the signal in "transposed" column
layout (prepared on host) and W_q[p, r] = g[128*(q-Q0) + p - r + c].

Mixed precision, tuned so worst-case rel err stays under the 2e-2
gate (measured 1.655e-2, deterministic; validated bit-exactly by a
host-side quantization simulator): inner taps run in fp16 (1 col/cycle
on the PE, 4x less quantization noise than bf16 for free); outer taps
run as fp8e4 DoubleRow pairs (256-wide contraction per instruction =
2 chunks per ~1.1*512 cycles, ~1.9x fp16 throughput). pha: 5 fp16
chunks (q' in [-2,2]) + 4 fp8 pairs ((-6,-5),(-4,-3),(+3,+4),(+5,+6));
amp: 1 fp16 chunk (q'=+1) + 1 fp8 pair ((-1,0)). Every DR pair is
served by ONE shift-1 stacked fp8 signal copy (xd8), so the only
device inputs are xt (fp16 signal), xd8, and the weight chunks.
Output staged/stored fp16.

Schedule (all measured on HW traces): ~48 zero-weight N=128 warmup
matmuls (no DMA deps) keep the PE busy through the HAM activity window
during the initial DMA wait so real matmuls run at 2.4 GHz from the
start; the three DMA rings (sync=xt, scalar=weights, gpsimd=xd8) each
lead with exactly what the first matmuls need (the rings share ~350
GB/s, and per-DMA costs ~0.65us sequencer issue + ~2us completion
latency, so band weights go as per-band slices in consumption order);
PSUM drains alternate vector/scalar into a per-band fp16 stage stored
as one 512KB DMA. The final band tapers its last groups (512,512,512,
256,128,128 cols) with parallel half-copies and sync/scalar-alternating
stores so the last store's copy+issue+data+sem chain is minimal; the
gpsimd ring gets no late DMAs (its end-of-program drain takes ~3.5us).
"""
import numpy as np
import ml_dtypes

import concourse.bass as bass
import concourse.tile as tile
from concourse import bacc, mybir
from concourse import bass_utils

# ---- problem geometry (hardcoded per contest rules) ----
T = 2097152
NCORES = 8
L = T // NCORES          # 262144 samples per core
LC = L // 128            # 2048 output columns per core
XH = 6                   # x halo columns each side (= pha (K-1)/128)
XC = LC + 2 * XH         # 2060 x columns
NB = 10                  # bands per filter group
QP, Q0P = 13, 6          # pha fused autocorr (1537 taps): chunk count, offset
QA, Q0A = 3, 1           # amp fused autocorr (257 taps): chunk count, offset
PB = 5                   # pha fp16 chunks per band (q' in [-2, 2])
NP8 = 8                  # pha fp8 chunks per band (q' = -6..-3, +3..+6)
P8OFF = (0, 2, 9, 11)    # xd8 base offsets of the 4 fp8 pairs
CP = 384                 # pha edge-splice width ((K-1)/2)
CA = 64                  # amp edge-splice width
N = 512                  # matmul moving width (1 PSUM bank)
NG = LC // N             # 4 groups per band
NDUMMY = 48              # PE warmup matmuls (N=128)

F32 = mybir.dt.float32
FP16 = mybir.dt.float16
FP8 = mybir.dt.float8e4
FP16_NP = np.float16
FP8_NP = ml_dtypes.float8_e4m3
DR = mybir.MatmulPerfMode.DoubleRow


def _toeplitz_chunks(g, Q0, NQ):
    """W[q][p, r] = g[128*(q - Q0) + p - r + c], zero outside [0, len(g))."""
    g = np.asarray(g, np.float64)
    K = len(g)
    c = (K - 1) // 2
    W = np.zeros((NQ, 128, 128), np.float64)
    p = np.arange(128)[:, None]
    r = np.arange(128)[None, :]
    for q in range(NQ):
        k = 128 * (q - Q0) + p - r + c
        valid = (k >= 0) & (k < K)
        W[q][valid] = g[np.clip(k, 0, K - 1)][valid]
    return W


def _build_program():
    nc = bacc.Bacc("TRN2", target_bir_lowering=False, debug=False,
                   enable_asserts=False, num_devices=NCORES)

    x_ap = nc.dram_tensor("xT", [128, XC], FP16, kind="ExternalInput").ap()
    xd_ap = nc.dram_tensor("xd8", [128, 2, XC], FP8, kind="ExternalInput").ap()
    wp_ap = nc.dram_tensor("wp", [128, NB * PB * 128], FP16,
                           kind="ExternalInput").ap()
    wp8_ap = nc.dram_tensor("wp8", [128, NB * NP8, 128], FP8,
                            kind="ExternalInput").ap()
    wa_ap = nc.dram_tensor("wa", [128, NB * 128], FP16,
                           kind="ExternalInput").ap()
    wa8_ap = nc.dram_tensor("wa8", [128, NB * 2, 128], FP8,
                            kind="ExternalInput").ap()
    out_ap = nc.dram_tensor("out", [2 * NB, 128, LC], FP16,
                            kind="ExternalOutput").ap()

    with tile.TileContext(nc) as tc:
        with tc.tile_pool(name="const", bufs=1) as cpool, \
             tc.tile_pool(name="psum", bufs=8, space="PSUM") as psum_pool, \
             tc.tile_pool(name="stage", bufs=8) as stage_pool:

            xt = cpool.tile([128, XC], FP16, name="xt", tag="xT")
            xd8 = cpool.tile([128, 2, XC], FP8, name="xdt", tag="xd8")
            wp = cpool.tile([128, NB * PB * 128], FP16, name="wpt", tag="wp")
            wp8 = cpool.tile([128, NB * NP8, 128], FP8, name="wp8t", tag="wp8")
            wa = cpool.tile([128, NB * 128], FP16, name="wat", tag="wa")
            wa8 = cpool.tile([128, NB * 2, 128], FP8, name="wa8t", tag="wa8")
            zt = cpool.tile([128, 128], FP16, name="zt", tag=None)

            # PE warmup: zero-weight matmuls with no DMA dependencies keep
            # the PE busy through the HAM activity window during the
            # startup DMA wait, so real matmuls run at 2.4 GHz from the
            # first instruction.
            nc.vector.memset(zt[:], 0.0)
            dps = psum_pool.tile([128, 512], F32, tag="ps")
            for _ in range(NDUMMY):
                nc.tensor.matmul(dps[:, 0:128], zt[:], zt[:],
                                 start=True, stop=True)

            # Startup-critical DMAs, one ring each so they land together.
            # The first real matmul (band-0 g0, fp16 chunk j=0) gates on
            # 33 KB of weights (scalar) + 133 KB of xt (sync); each ring
            # leads with exactly what the earliest matmuls need.
            nc.sync.dma_start(xt[:, 0:520], x_ap[:, 0:520])
            nc.scalar.dma_start(wp[:, 0:128], wp_ap[:, 0:128])
            nc.gpsimd.dma_start(xd8[:, :, 0:520], xd_ap[:, :, 0:520])
            nc.sync.dma_start(xt[:, 520:1040], x_ap[:, 520:1040])
            nc.scalar.dma_start(wp[:, 128:PB * 128], wp_ap[:, 128:PB * 128])
            nc.gpsimd.dma_start(xd8[:, :, 520:1040], xd_ap[:, :, 520:1040])
            nc.sync.dma_start(xt[:, 1040:1560], x_ap[:, 1040:1560])
            nc.scalar.dma_start(wp8[:, 0:NP8, :], wp8_ap[:, 0:NP8, :])
            nc.gpsimd.dma_start(xd8[:, :, 1040:1560], xd_ap[:, :, 1040:1560])
            nc.sync.dma_start(xt[:, 1560:XC], x_ap[:, 1560:XC])
            nc.scalar.dma_start(wa[:, 0:128], wa_ap[:, 0:128])
            nc.gpsimd.dma_start(xd8[:, :, 1560:XC], xd_ap[:, :, 1560:XC])
            nc.scalar.dma_start(wa8[:, 0:2, :], wa8_ap[:, 0:2, :])

            # bands 1-9 weights: per-band slices in consumption order so
            # each band's wait clears as soon as its own bytes land (the
            # input stream saturates DMA bandwidth for the first ~15us;
            # one big DMA would gate band 1 on band 9's bytes). Odd bands
            # ride the scalar ring, even bands gpsimd; fp8 before fp16
            # within a band pair matches nothing in-group (fp16 runs
            # first) but keeps the tighter-deadline DR weights earlier.
            def wband(eng, b):
                eng.dma_start(wp8[:, b * NP8:(b + 1) * NP8, :],
                              wp8_ap[:, b * NP8:(b + 1) * NP8, :])
                eng.dma_start(wp[:, b * PB * 128:(b + 1) * PB * 128],
                              wp_ap[:, b * PB * 128:(b + 1) * PB * 128])

            wband(nc.scalar, 1)
            nc.gpsimd.dma_start(wa[:, 128:], wa_ap[:, 128:])
            nc.gpsimd.dma_start(wa8[:, 2:, :], wa8_ap[:, 2:, :])
            for b in range(2, NB):
                wband(nc.scalar if b % 2 else nc.gpsimd, b)

            def pha_group(ps, b, i0, c0, w, first, stop):
                # one accumulation window over psum cols [c0, c0+w)
                for j in range(PB):
                    m0 = i0 + c0 + 4 + j
                    nc.tensor.matmul(
                        ps[:, c0:c0 + w],
                        wp[:, (b * PB + j) * 128:(b * PB + j + 1) * 128],
                        xt[:, m0:m0 + w],
                        start=(first and j == 0), stop=False)
                # fp8 pairs: (-6,-5), (-4,-3), (+3,+4), (+5,+6)
                for k, off in enumerate(P8OFF):
                    nc.tensor.matmul(
                        ps[:, c0:c0 + w],
                        wp8[:, b * NP8 + 2 * k:b * NP8 + 2 * k + 2, :],
                        xd8[:, :, i0 + c0 + off:i0 + c0 + off + w],
                        start=False, stop=(stop and k == 3), perf_mode=DR)

            def pha_band(b, last=False):
                if last:
                    # final band: taper the last groups so the very last
                    # store's copy+issue+data+sem chain is minimal; parallel
                    # half-copies, stores alternate sync/scalar rings (never
                    # gpsimd: its end-of-program queue drain takes ~3.5us).
                    widths = (N, N, N, N // 2, N // 4, N // 4)
                    i0 = 0
                    for g, w in enumerate(widths):
                        ps = psum_pool.tile([128, N], F32, tag="ps")
                        pha_group(ps, b, i0, 0, w, True, True)
                        st = stage_pool.tile([128, N], FP16, tag="stl")
                        h = w // 2
                        nc.vector.tensor_copy(st[:, 0:h], ps[:, 0:h])
                        nc.scalar.copy(st[:, h:w], ps[:, h:w])
                        eng = (nc.sync, nc.scalar)[g % 2]
                        eng.dma_start(out_ap[b, :, i0:i0 + w], st[:, 0:w])
                        i0 += w
                    return
                st = None
                for g in range(NG):
                    ps = psum_pool.tile([128, N], F32, tag="ps")
                    pha_group(ps, b, g * N, 0, N, True, True)
                    st = _drain(st, ps, g, b, False)

            def amp_group(b, i0, w):
                ps = psum_pool.tile([128, N], F32, tag="ps")
                # fp16 chunk q' = +1
                nc.tensor.matmul(
                    ps[:, 0:w], wa[:, b * 128:(b + 1) * 128],
                    xt[:, i0 + 7:i0 + 7 + w],
                    start=True, stop=False)
                # fp8 pair: chunks q' = -1, 0 (rhs blocks i-1, i) —
                # served by the same shift-1 stacked fp8 copy as pha
                nc.tensor.matmul(
                    ps[:, 0:w], wa8[:, b * 2:b * 2 + 2, :],
                    xd8[:, :, i0 + 5:i0 + 5 + w],
                    start=False, stop=True, perf_mode=DR)
                return ps

            def amp_band(b):
                st = None
                for g in range(NG):
                    ps = amp_group(b, g * N, N)
                    st = _drain(st, ps, g, NB + b, False)

            def _drain(st, ps, g, out_b, last):
                # all 4 groups of a band share one 2048-col fp16 stage ->
                # one 512KB store per band with 4KB DMA lines (the final
                # band is handled by pha_band's taper path instead)
                i0 = g * N
                if g == 0:
                    st = stage_pool.tile([128, LC], FP16, tag="st")
                if g % 2 == 0:
                    nc.vector.tensor_copy(st[:, i0:i0 + N], ps[:])
                else:
                    nc.scalar.copy(st[:, i0:i0 + N], ps[:])
                if g == NG - 1:
                    nc.sync.dma_start(out_ap[out_b, :, :], st[:])
                return st

            # amp interleaved between pha bands: amp's 2-matmul groups
            # produce drains ~4x faster than pha's 9-matmul groups; the
            # mix keeps DVE/ACT drain demand under their throughput. pha
            # runs first (its slow groups ride out the staggered xt/xd8
            # segment completion-sems at startup); the last pair is
            # swapped so the TAPERED pha band ends the program — its
            # widely-spaced drains leave the store rings idle, keeping
            # the final store chain off any queue.
            for b in range(NB - 1):
                pha_band(b)
                amp_band(b)
            amp_band(NB - 1)
            pha_band(NB - 1, last=True)

    nc.compile()
    return nc


_CACHE = {}


def _get_program():
    if "nc" not in _CACHE:
        _CACHE["nc"] = _build_program()
    return _CACHE["nc"]


def _host_inputs(x, pha_filters, amp_filters):
    x = np.ascontiguousarray(np.asarray(x, np.float32).reshape(T))
    pha = np.asarray(pha_filters, np.float64)
    amp = np.asarray(amp_filters, np.float64)

    gp = [np.correlate(h, h, "full") for h in pha]   # 1537 taps
    ga = [np.correlate(h, h, "full") for h in amp]   # 257 taps
    Wp = np.stack([_toeplitz_chunks(g, Q0P, QP) for g in gp])  # (NB,13,128,128)
    Wa = np.stack([_toeplitz_chunks(g, Q0A, QA) for g in ga])  # (NB,3,128,128)

    def wlay(W, dt):  # (NB, NQ, 128p, 128r) -> (128p, NB*NQ*128r)
        return np.ascontiguousarray(
            W.transpose(2, 0, 1, 3).reshape(128, -1).astype(dt))

    wp = wlay(Wp[:, 4:9], FP16_NP)                       # q' in [-2, 2]
    wp8 = wlay(Wp[:, [0, 1, 2, 3, 9, 10, 11, 12]], FP8_NP)  # q' = +-3..+-6
    wa = wlay(Wa[:, 2:3], FP16_NP)                       # q' = +1
    wa8 = wlay(Wa[:, [0, 1]], FP8_NP)                    # q' = -1, 0

    xpad = np.zeros(T + (2 * XH + 2) * 128, np.float32)
    xpad[XH * 128: XH * 128 + T] = x

    in_maps = []
    for c in range(NCORES):
        n0 = c * L
        xcols = xpad[n0:n0 + (XC + 2) * 128].reshape(XC + 2, 128).T
        xt = np.ascontiguousarray(xcols[:, :XC].astype(FP16_NP))
        x8 = xcols.astype(FP8_NP)
        xd8 = np.ascontiguousarray(
            np.stack([x8[:, 0:XC], x8[:, 1:XC + 1]], axis=1))
        in_maps.append({"xT": xt, "xd8": xd8,
                        "wp": wp, "wp8": wp8, "wa": wa, "wa8": wa8})
    return in_maps


def _edge_exact(x, h, W, win=3072):
    """Exact two-pass values for out[:W] and out[T-W:] (float64 host)."""
    K = len(h)
    c = (K - 1) // 2
    xs = x[:win]
    f1 = np.convolve(xs, h[::-1])
    y1 = f1[K - 1 - c:K - 1 - c + win]
    y1v = win - c
    f2 = np.convolve(y1[:y1v], h)
    head = f2[c:c + y1v - c][:W]
    xs = x[::-1][:win]
    f1 = np.convolve(xs, h)
    y1r = f1[c:c + win]
    f2 = np.convolve(y1r[:y1v], h[::-1])
    tail = f2[K - 1 - c:K - 1 - c + y1v - c][:W][::-1]
    return head, tail


def _gather(results, x, pha_filters, amp_filters):
    out = np.empty((2 * NB, T), np.float32)
    for c in range(NCORES):
        oc = np.asarray(results[c]["out"]).astype(np.float32)
        out[:, c * L:(c + 1) * L] = oc.transpose(0, 2, 1).reshape(2 * NB, L)
    # splice exact global-edge samples (fused autocorr differs from the
    # reference's cropped two-pass only within (K-1)/2 of each end)
    x64 = np.asarray(x, np.float64).reshape(T)
    for b in range(NB):
        head, tail = _edge_exact(x64, np.asarray(pha_filters[b], np.float64), CP)
        out[b, :CP] = head
        out[b, T - CP:] = tail
        head, tail = _edge_exact(x64, np.asarray(amp_filters[b], np.float64), CA)
        out[NB + b, :CA] = head
        out[NB + b, T - CA:] = tail
    return out.reshape(1, 1, 2 * NB * T)


def run(x, pha_filters, amp_filters, trace=False):
    nc = _get_program()
    in_maps = _host_inputs(x, pha_filters, amp_filters)
    res = bass_utils.run_bass_kernel_spmd(
        nc, in_maps, core_ids=list(range(NCORES)), trace=trace)
    return _gather(res.results, x, pha_filters, amp_filters), res


def kernel(x, pha_filters, amp_filters):
    out, _ = run(x, pha_filters, amp_filters)
    return out


# revision 36
# speedup vs baseline: 1.1958x; 1.0015x over previous
"""Trainium2 Bass kernel for CombinedBandPassFilterSequential.

Zero-phase (filtfilt-style) FIR filter bank: 10 phase bands (K=769) +
10 amplitude bands (K=129) over a single (1,1,2097152) fp32 signal;
output is the 20 band signals concatenated on the last axis.

Strategy
--------
Time-sharded SPMD over 8 NeuronCores: each core processes a contiguous
T/8 slice of the signal for ALL 20 bands (perfect load balance).

The two-pass zero-phase filter equals a single cross-correlation with
g = autocorr(h) (2K-1 taps) everywhere except the first/last (K-1)/2
samples of the GLOBAL sequence; those few samples are computed exactly
on the host (numpy, float64) and spliced into the result. This fuses
the reference's two conv passes into one dense pass with no
intermediate staging and no edge masks on the device.

Each 1-D correlation is cast as a sequence of 128x128 @ 128x512
tensor-engine matmuls using banded-Toeplitz weight chunks
(PSUM accumulates fp32):

  out[128*i + r] = sum_q  W_q[:, r] . x_cols[:, i + q - Q0]

where x_cols[p, m] = x[128*m + p] is the signal in "transposed" column
layout (prepared on host) and W_q[p, r] = g[128*(q-Q0) + p - r + c].

Mixed precision, tuned so worst-case rel err stays under the 2e-2
gate (measured 1.655e-2, deterministic; validated bit-exactly by a
host-side quantization simulator): inner taps run in fp16 (1 col/cycle
on the PE, 4x less quantization noise than bf16 for free); outer taps
run as fp8e4 DoubleRow pairs (256-wide contraction per instruction =
2 chunks per ~1.1*512 cycles, ~1.9x fp16 throughput). pha: 5 fp16
chunks (q' in [-2,2]) + 4 fp8 pairs ((-6,-5),(-4,-3),(+3,+4),(+5,+6));
amp: 1 fp16 chunk (q'=+1) + 1 fp8 pair ((-1,0)). Every DR pair is
served by ONE shift-1 stacked fp8 signal copy (xd8), so the only
device inputs are xt (fp16 signal), xd8, and the weight chunks.
Output staged/stored fp16.

Schedule (all measured on HW traces): ~48 zero-weight N=128 warmup
matmuls (no DMA deps) keep the PE busy through the HAM activity window
during the initial DMA wait so real matmuls run at 2.4 GHz from the
start; the three DMA rings (sync=xt, scalar=weights, gpsimd=xd8) each
lead with exactly what the first matmuls need (the rings share ~350
GB/s, and per-DMA costs ~0.65us sequencer issue + ~2us completion
latency, so band weights go as per-band slices in consumption order);
PSUM drains alternate vector/scalar into a per-band fp16 stage stored
as one 512KB DMA. The final band tapers its last groups (512,512,512,
256,128,128 cols) with parallel half-copies and sync/scalar-alternating
stores so the last store's copy+issue+data+sem chain is minimal; the
gpsimd ring gets no late DMAs (its end-of-program drain takes ~3.5us).
"""
import numpy as np
import ml_dtypes

import concourse.bass as bass
import concourse.tile as tile
from concourse import bacc, mybir
from concourse import bass_utils

# ---- problem geometry (hardcoded per contest rules) ----
T = 2097152
NCORES = 8
L = T // NCORES          # 262144 samples per core
LC = L // 128            # 2048 output columns per core
XH = 6                   # x halo columns each side (= pha (K-1)/128)
XC = LC + 2 * XH         # 2060 x columns
NB = 10                  # bands per filter group
QP, Q0P = 13, 6          # pha fused autocorr (1537 taps): chunk count, offset
QA, Q0A = 3, 1           # amp fused autocorr (257 taps): chunk count, offset
PB = 5                   # pha fp16 chunks per band (q' in [-2, 2])
NP8 = 8                  # pha fp8 chunks per band (q' = -6..-3, +3..+6)
P8OFF = (0, 2, 9, 11)    # xd8 base offsets of the 4 fp8 pairs
CP = 384                 # pha edge-splice width ((K-1)/2)
CA = 64                  # amp edge-splice width
N = 512                  # matmul moving width (1 PSUM bank)
NG = LC // N             # 4 groups per band
NDUMMY = 54              # PE warmup matmuls (N=128)

F32 = mybir.dt.float32
FP16 = mybir.dt.float16
FP8 = mybir.dt.float8e4
FP16_NP = np.float16
FP8_NP = ml_dtypes.float8_e4m3
DR = mybir.MatmulPerfMode.DoubleRow


def _toeplitz_chunks(g, Q0, NQ):
    """W[q][p, r] = g[128*(q - Q0) + p - r + c], zero outside [0, len(g))."""
    g = np.asarray(g, np.float64)
    K = len(g)
    c = (K - 1) // 2
    W = np.zeros((NQ, 128, 128), np.float64)
    p = np.arange(128)[:, None]
    r = np.arange(128)[None, :]
    for q in range(NQ):
        k = 128 * (q - Q0) + p - r + c
        valid = (k >= 0) & (k < K)
        W[q][valid] = g[np.clip(k, 0, K - 1)][valid]
    return W


def _build_program():
    nc = bacc.Bacc("TRN2", target_bir_lowering=False, debug=False,
                   enable_asserts=False, num_devices=NCORES)

    x_ap = nc.dram_tensor("xT", [128, XC], FP16, kind="ExternalInput").ap()
    xd_ap = nc.dram_tensor("xd8", [128, 2, XC], FP8, kind="ExternalInput").ap()
    wp_ap = nc.dram_tensor("wp", [128, NB * PB * 128], FP16,
                           kind="ExternalInput").ap()
    wp8_ap = nc.dram_tensor("wp8", [128, NB * NP8, 128], FP8,
                            kind="ExternalInput").ap()
    wa_ap = nc.dram_tensor("wa", [128, NB * 128], FP16,
                           kind="ExternalInput").ap()
    wa8_ap = nc.dram_tensor("wa8", [128, NB * 2, 128], FP8,
                            kind="ExternalInput").ap()
    out_ap = nc.dram_tensor("out", [2 * NB, 128, LC], FP16,
                            kind="ExternalOutput").ap()

    with tile.TileContext(nc) as tc:
        with tc.tile_pool(name="const", bufs=1) as cpool, \
             tc.tile_pool(name="psum", bufs=8, space="PSUM") as psum_pool, \
             tc.tile_pool(name="stage", bufs=8) as stage_pool:

            xt = cpool.tile([128, XC], FP16, name="xt", tag="xT")
            xd8 = cpool.tile([128, 2, XC], FP8, name="xdt", tag="xd8")
            wp = cpool.tile([128, NB * PB * 128], FP16, name="wpt", tag="wp")
            wp8 = cpool.tile([128, NB * NP8, 128], FP8, name="wp8t", tag="wp8")
            wa = cpool.tile([128, NB * 128], FP16, name="wat", tag="wa")
            wa8 = cpool.tile([128, NB * 2, 128], FP8, name="wa8t", tag="wa8")
            zt = cpool.tile([128, 128], FP16, name="zt", tag=None)

            # PE warmup: zero-weight matmuls with no DMA dependencies keep
            # the PE busy through the HAM activity window during the
            # startup DMA wait, so real matmuls run at 2.4 GHz from the
            # first instruction.
            nc.vector.memset(zt[:], 0.0)
            dps = psum_pool.tile([128, 512], F32, tag="ps")
            for _ in range(NDUMMY):
                nc.tensor.matmul(dps[:, 0:128], zt[:], zt[:],
                                 start=True, stop=True)

            # Startup-critical DMAs, one ring each so they land together.
            # The first real matmul (band-0 g0, fp16 chunk j=0) gates on
            # 33 KB of weights (scalar) + 133 KB of xt (sync); each ring
            # leads with exactly what the earliest matmuls need.
            nc.sync.dma_start(xt[:, 0:520], x_ap[:, 0:520])
            nc.scalar.dma_start(wp[:, 0:128], wp_ap[:, 0:128])
            nc.gpsimd.dma_start(xd8[:, :, 0:520], xd_ap[:, :, 0:520])
            nc.sync.dma_start(xt[:, 520:1040], x_ap[:, 520:1040])
            nc.scalar.dma_start(wp[:, 128:PB * 128], wp_ap[:, 128:PB * 128])
            nc.gpsimd.dma_start(xd8[:, :, 520:1040], xd_ap[:, :, 520:1040])
            nc.sync.dma_start(xt[:, 1040:1560], x_ap[:, 1040:1560])
            nc.scalar.dma_start(wp8[:, 0:NP8, :], wp8_ap[:, 0:NP8, :])
            nc.gpsimd.dma_start(xd8[:, :, 1040:1560], xd_ap[:, :, 1040:1560])
            nc.sync.dma_start(xt[:, 1560:XC], x_ap[:, 1560:XC])
            nc.scalar.dma_start(wa[:, 0:128], wa_ap[:, 0:128])
            nc.gpsimd.dma_start(xd8[:, :, 1560:XC], xd_ap[:, :, 1560:XC])
            nc.scalar.dma_start(wa8[:, 0:2, :], wa8_ap[:, 0:2, :])

            # bands 1-9 weights: per-band slices in consumption order so
            # each band's wait clears as soon as its own bytes land (the
            # input stream saturates DMA bandwidth for the first ~15us;
            # one big DMA would gate band 1 on band 9's bytes). Odd bands
            # ride the scalar ring, even bands gpsimd; fp8 before fp16
            # within a band pair matches nothing in-group (fp16 runs
            # first) but keeps the tighter-deadline DR weights earlier.
            def wband(eng, b):
                eng.dma_start(wp8[:, b * NP8:(b + 1) * NP8, :],
                              wp8_ap[:, b * NP8:(b + 1) * NP8, :])
                eng.dma_start(wp[:, b * PB * 128:(b + 1) * PB * 128],
                              wp_ap[:, b * PB * 128:(b + 1) * PB * 128])

            wband(nc.scalar, 1)
            nc.gpsimd.dma_start(wa[:, 128:], wa_ap[:, 128:])
            nc.gpsimd.dma_start(wa8[:, 2:, :], wa8_ap[:, 2:, :])
            for b in range(2, NB):
                wband(nc.scalar if b % 2 else nc.gpsimd, b)

            def pha_group(ps, b, i0, c0, w, first, stop):
                # one accumulation window over psum cols [c0, c0+w)
                for j in range(PB):
                    m0 = i0 + c0 + 4 + j
                    nc.tensor.matmul(
                        ps[:, c0:c0 + w],
                        wp[:, (b * PB + j) * 128:(b * PB + j + 1) * 128],
                        xt[:, m0:m0 + w],
                        start=(first and j == 0), stop=False)
                # fp8 pairs: (-6,-5), (-4,-3), (+3,+4), (+5,+6)
                for k, off in enumerate(P8OFF):
                    nc.tensor.matmul(
                        ps[:, c0:c0 + w],
                        wp8[:, b * NP8 + 2 * k:b * NP8 + 2 * k + 2, :],
                        xd8[:, :, i0 + c0 + off:i0 + c0 + off + w],
                        start=False, stop=(stop and k == 3), perf_mode=DR)

            def pha_band(b, last=False):
                if last:
                    # final band: taper the last groups so the very last
                    # store's copy+issue+data+sem chain is minimal; parallel
                    # half-copies, stores alternate sync/scalar rings (never
                    # gpsimd: its end-of-program queue drain takes ~3.5us).
                    widths = (N, N, N, N // 2, N // 4, N // 4)
                    i0 = 0
                    for g, w in enumerate(widths):
                        ps = psum_pool.tile([128, N], F32, tag="ps")
                        pha_group(ps, b, i0, 0, w, True, True)
                        st = stage_pool.tile([128, N], FP16, tag="stl")
                        h = w // 2
                        nc.vector.tensor_copy(st[:, 0:h], ps[:, 0:h])
                        nc.scalar.copy(st[:, h:w], ps[:, h:w])
                        eng = (nc.sync, nc.scalar)[g % 2]
                        eng.dma_start(out_ap[b, :, i0:i0 + w], st[:, 0:w])
                        i0 += w
                    return
                st = None
                for g in range(NG):
                    ps = psum_pool.tile([128, N], F32, tag="ps")
                    pha_group(ps, b, g * N, 0, N, True, True)
                    st = _drain(st, ps, g, b, False)

            def amp_group(b, i0, w):
                ps = psum_pool.tile([128, N], F32, tag="ps")
                # fp16 chunk q' = +1
                nc.tensor.matmul(
                    ps[:, 0:w], wa[:, b * 128:(b + 1) * 128],
                    xt[:, i0 + 7:i0 + 7 + w],
                    start=True, stop=False)
                # fp8 pair: chunks q' = -1, 0 (rhs blocks i-1, i) —
                # served by the same shift-1 stacked fp8 copy as pha
                nc.tensor.matmul(
                    ps[:, 0:w], wa8[:, b * 2:b * 2 + 2, :],
                    xd8[:, :, i0 + 5:i0 + 5 + w],
                    start=False, stop=True, perf_mode=DR)
                return ps

            def amp_band(b):
                st = None
                for g in range(NG):
                    ps = amp_group(b, g * N, N)
                    st = _drain(st, ps, g, NB + b, False)

            def _drain(st, ps, g, out_b, last):
                # all 4 groups of a band share one 2048-col fp16 stage ->
                # one 512KB store per band with 4KB DMA lines (the final
                # band is handled by pha_band's taper path instead)
                i0 = g * N
                if g == 0:
                    st = stage_pool.tile([128, LC], FP16, tag="st")
                if g % 2 == 0:
                    nc.vector.tensor_copy(st[:, i0:i0 + N], ps[:])
                else:
                    nc.scalar.copy(st[:, i0:i0 + N], ps[:])
                if g == NG - 1:
                    nc.sync.dma_start(out_ap[out_b, :, :], st[:])
                return st

            # amp interleaved between pha bands: amp's 2-matmul groups
            # produce drains ~4x faster than pha's 9-matmul groups; the
            # mix keeps DVE/ACT drain demand under their throughput. pha
            # runs first (its slow groups ride out the staggered xt/xd8
            # segment completion-sems at startup); the last pair is
            # swapped so the TAPERED pha band ends the program — its
            # widely-spaced drains leave the store rings idle, keeping
            # the final store chain off any queue.
            for b in range(NB - 1):
                pha_band(b)
                amp_band(b)
            amp_band(NB - 1)
            pha_band(NB - 1, last=True)

    nc.compile()
    return nc


_CACHE = {}


def _get_program():
    if "nc" not in _CACHE:
        _CACHE["nc"] = _build_program()
    return _CACHE["nc"]


def _host_inputs(x, pha_filters, amp_filters):
    x = np.ascontiguousarray(np.asarray(x, np.float32).reshape(T))
    pha = np.asarray(pha_filters, np.float64)
    amp = np.asarray(amp_filters, np.float64)

    gp = [np.correlate(h, h, "full") for h in pha]   # 1537 taps
    ga = [np.correlate(h, h, "full") for h in amp]   # 257 taps
    Wp = np.stack([_toeplitz_chunks(g, Q0P, QP) for g in gp])  # (NB,13,128,128)
    Wa = np.stack([_toeplitz_chunks(g, Q0A, QA) for g in ga])  # (NB,3,128,128)

    def wlay(W, dt):  # (NB, NQ, 128p, 128r) -> (128p, NB*NQ*128r)
        return np.ascontiguousarray(
            W.transpose(2, 0, 1, 3).reshape(128, -1).astype(dt))

    wp = wlay(Wp[:, 4:9], FP16_NP)                       # q' in [-2, 2]
    wp8 = wlay(Wp[:, [0, 1, 2, 3, 9, 10, 11, 12]], FP8_NP)  # q' = +-3..+-6
    wa = wlay(Wa[:, 2:3], FP16_NP)                       # q' = +1
    wa8 = wlay(Wa[:, [0, 1]], FP8_NP)                    # q' = -1, 0

    xpad = np.zeros(T + (2 * XH + 2) * 128, np.float32)
    xpad[XH * 128: XH * 128 + T] = x

    in_maps = []
    for c in range(NCORES):
        n0 = c * L
        xcols = xpad[n0:n0 + (XC + 2) * 128].reshape(XC + 2, 128).T
        xt = np.ascontiguousarray(xcols[:, :XC].astype(FP16_NP))
        x8 = xcols.astype(FP8_NP)
        xd8 = np.ascontiguousarray(
            np.stack([x8[:, 0:XC], x8[:, 1:XC + 1]], axis=1))
        in_maps.append({"xT": xt, "xd8": xd8,
                        "wp": wp, "wp8": wp8, "wa": wa, "wa8": wa8})
    return in_maps


def _edge_exact(x, h, W, win=3072):
    """Exact two-pass values for out[:W] and out[T-W:] (float64 host)."""
    K = len(h)
    c = (K - 1) // 2
    xs = x[:win]
    f1 = np.convolve(xs, h[::-1])
    y1 = f1[K - 1 - c:K - 1 - c + win]
    y1v = win - c
    f2 = np.convolve(y1[:y1v], h)
    head = f2[c:c + y1v - c][:W]
    xs = x[::-1][:win]
    f1 = np.convolve(xs, h)
    y1r = f1[c:c + win]
    f2 = np.convolve(y1r[:y1v], h[::-1])
    tail = f2[K - 1 - c:K - 1 - c + y1v - c][:W][::-1]
    return head, tail


def _gather(results, x, pha_filters, amp_filters):
    out = np.empty((2 * NB, T), np.float32)
    for c in range(NCORES):
        oc = np.asarray(results[c]["out"]).astype(np.float32)
        out[:, c * L:(c + 1) * L] = oc.transpose(0, 2, 1).reshape(2 * NB, L)
    # splice exact global-edge samples (fused autocorr differs from the
    # reference's cropped two-pass only within (K-1)/2 of each end)
    x64 = np.asarray(x, np.float64).reshape(T)
    for b in range(NB):
        head, tail = _edge_exact(x64, np.asarray(pha_filters[b], np.float64), CP)
        out[b, :CP] = head
        out[b, T - CP:] = tail
        head, tail = _edge_exact(x64, np.asarray(amp_filters[b], np.float64), CA)
        out[NB + b, :CA] = head
        out[NB + b, T - CA:] = tail
    return out.reshape(1, 1, 2 * NB * T)


def run(x, pha_filters, amp_filters, trace=False):
    nc = _get_program()
    in_maps = _host_inputs(x, pha_filters, amp_filters)
    res = bass_utils.run_bass_kernel_spmd(
        nc, in_maps, core_ids=list(range(NCORES)), trace=trace)
    return _gather(res.results, x, pha_filters, amp_filters), res


def kernel(x, pha_filters, amp_filters):
    out, _ = run(x, pha_filters, amp_filters)
    return out
